# revision 36
# baseline (speedup 1.0000x reference)
"""GCN (3-layer) on Trainium2, 8-core SPMD.

Strategy:
  - Shard destination nodes across cores (each core owns N/NC contiguous dst).
  - Per layer: each core computes its slice of the scaled feature table
    p = dinv * (h @ W)   (or p = dinv * h for layer 3),
    AllGather -> replicated table [GLOBAL, FH] in each core's DRAM.
  - Aggregation (pull): per src-window (int16 gather limit 32768 rows),
    dma_gather edge messages in a fixed-degree-class layout, DVE strided
    reduce -> per-window partial tables T_w (window bucket order).
  - Stage 2: gather each node's NWIN partials (canonical order) and sum.
  - out = relu(dinv * agg + b); layer 3: z = dinv*agg3; out = log_softmax(z@W3+b3).

Performance-critical details (measured on HW, not just the cost model):
  - All gathers fetch 512B descriptors (elem_size=128 f32, elem_step=64:
    rows r and r+1 in one overlapping-pair fetch; only the left half is
    consumed).  256B descriptors run ~2.4x slower per byte on HW.
  - 4 SWDGE queues (ucode max) ~2x gather throughput over 2.
  - Reduce outputs are packed per bucket-tile pair so Tw writes are
    512B/partition runs; stage-2 idx arrays address the packed layout.
  - Epilogues are batched over 14-tile chunks (fewer instructions).

All row orderings/permutations are baked into host-precomputed int16 index
arrays; the device program is identical on all cores (SPMD), only data differs.
"""
import numpy as np
import sys

sys.path.insert(0, "/opt/trn_rl_repo")
sys.path.insert(0, "/opt/trn_rl_repo/concourse")

import bass_rust  # noqa: E402
from concourse import mybir, bacc, tile  # noqa: E402
from concourse.masks import make_identity  # noqa: E402


def _with_ap(ap_obj, pattern):
    """Copy of an AP with a custom [[stride, size], ...] pattern.

    Used to build overlapping-row gather sources (elem_size=128 f32 rows
    fetched with elem_step=64: one 512B DMA descriptor covers table rows
    r and r+1, which runs ~2.4x faster per byte than 256B descriptors)
    and to build half-width strided views of the gathered buffers.
    """
    c = ap_obj.copy()
    c.ap = bass_rust.VecI64Pair(pattern)
    return c


def wrap16(arr):
    """[M] int -> [128, M/16] int16 layout for dma_gather idx tiles
    (idx j at partition j%16, col j//16, replicated to all 8 groups)."""
    M = arr.shape[0]
    assert M % 16 == 0
    out = np.zeros((128, M // 16), dtype=np.int16)
    pat = arr.reshape(M // 16, 16).T.astype(np.int16)  # [16, M/16]
    for r in range(8):
        out[r * 16:(r + 1) * 16, :] = pat
    return out


def preprocess(x, edge_index, n_cores=8, nwin=None):
    """Host-side graph preprocessing. Returns (meta, in_maps)."""
    N, F_IN = x.shape
    src = np.asarray(edge_index[0], dtype=np.int64)
    dst = np.asarray(edge_index[1], dtype=np.int64)
    # self loops
    loops = np.arange(N, dtype=np.int64)
    src = np.concatenate([src, loops])
    dst = np.concatenate([dst, loops])

    NC = n_cores
    assert N % NC == 0
    LR = N // NC                       # real nodes per core
    LOCAL = ((LR // 128) + 1) * 128 if LR % 128 == 0 else ((LR + 127) // 128) * 128
    if LOCAL == LR:
        LOCAL += 128
    NT = LOCAL // 128
    GLOBAL = NC * LOCAL
    if nwin is None:
        nwin = 1
        while GLOBAL // nwin > 32767 or GLOBAL % nwin:
            nwin += 1
    NWIN = nwin
    assert LOCAL % NWIN == 0
    QL = LOCAL // NWIN
    # table is quarter-major: window w = [w*WR, w*WR+NC*QL) + 128 zero-pad rows
    WR = NC * QL + 128
    assert WR <= 32767

    deg = np.bincount(dst, minlength=N).astype(np.int64)  # includes self loop

    # quarter-major in-window row of each edge's src:
    #   src core c, local l -> window l//QL, in-window row c*QL + l%QL
    c_src = src // LR
    l_src = src % LR
    win_src = l_src // QL
    row_src = c_src * QL + (l_src % QL)
    # zero-pad rows sit at the tail of every window's table region
    PAD_OFF = np.full(NWIN, NC * QL, dtype=np.int64)

    core_of_edge = dst // LR
    win_of_edge = win_src

    # per-core, per-window adjacency (local dst -> list of in-window src rows)
    # sort by (core, window, local dst, src-row): lexsort primary key last
    order_all = np.lexsort((row_src, dst % LR, win_of_edge, core_of_edge))
    s_sorted = row_src[order_all]
    w_sorted = win_of_edge[order_all]
    d_sorted = dst[order_all]

    # d_w counts per (core, window, local node)
    dw = np.zeros((NC, NWIN, LOCAL), dtype=np.int32)
    np.add.at(dw, (core_of_edge[order_all], w_sorted, d_sorted % LR), 1)

    # bucket order per (core, window): nodes sorted by d_w ascending (stable)
    border = np.zeros((NC, NWIN, LOCAL), dtype=np.int64)   # bucket pos -> local node
    bpos = np.zeros((NC, NWIN, LOCAL), dtype=np.int64)     # local node -> bucket pos
    for c in range(NC):
        for w in range(NWIN):
            o = np.argsort(dw[c, w], kind="stable")
            border[c, w] = o
            bpos[c, w][o] = np.arange(LOCAL)

    # uniform tile classes DW[w][t] = max over cores of tile max (>=1)
    DW = np.zeros((NWIN, NT), dtype=np.int64)
    for w in range(NWIN):
        for t in range(NT):
            m = 0
            for c in range(NC):
                nodes = border[c, w, t * 128:(t + 1) * 128]
                m = max(m, int(dw[c, w][nodes].max()))
            DW[w, t] = max(1, m)

    # per-call slot offsets (in idx elements) within each window's list
    cumD = np.zeros((NWIN, NT + 1), dtype=np.int64)
    for w in range(NWIN):
        cumD[w, 1:] = np.cumsum(128 * DW[w])

    # within-node rank of each edge in the sorted list (vectorized)
    key = (core_of_edge[order_all] * NWIN + w_sorted) * LOCAL + (d_sorted % LR)
    run_start = np.zeros(len(key), dtype=np.int64)
    newrun = np.ones(len(key), dtype=bool)
    newrun[1:] = key[1:] != key[:-1]
    run_idx = np.flatnonzero(newrun)
    run_start[run_idx] = run_idx
    run_start = np.maximum.accumulate(run_start)
    rank = np.arange(len(key)) - run_start                       # k per edge

    l_all = d_sorted % LR
    c_all = core_of_edge[order_all]
    # bucket position of the edge's dst in its (c, w) ordering
    bp_all = bpos[c_all, w_sorted, l_all]
    t_all = bp_all // 128
    p_all = bp_all % 128
    slot = cumD[w_sorted, t_all] + rank * 128 + p_all            # pos in window list

    idx1 = [[None] * NWIN for _ in range(NC)]
    for c in range(NC):
        for w in range(NWIN):
            m = (c_all == c) & (w_sorted == w)
            n_slots = int(cumD[w, NT])
            # spread pad fetches over 96 zero rows (avoid an HBM hotspot;
            # +1 overlap row stays inside the 128-row zero block)
            buf = PAD_OFF[w] + (np.arange(n_slots, dtype=np.int64) % 96)
            buf[slot[m]] = s_sorted[m]
            idx1[c][w] = wrap16(buf)

    # idx1 per-call column offsets (in int16 cols = NI/16)
    off1 = cumD // 16

    # stage2: idx2[c][w][i] = STORAGE row of canonical node i in window w.
    # Tw storage packs bucket-tile pairs: bucket row (t, p) lives at
    # storage row (t//2)*256 + 2p + t%2 (512B/partition Tw writes).
    idx2 = np.zeros((NC, NWIN, 128, LOCAL // 16), dtype=np.int16)
    for c in range(NC):
        for w in range(NWIN):
            bp = bpos[c, w]
            st = (bp // 256) * 256 + (bp % 128) * 2 + (bp // 128) % 2
            idx2[c, w] = wrap16(st)

    # deg table [128, NT] fp32 (pads = +inf)
    degt = np.zeros((NC, 128, NT), dtype=np.float32)
    for c in range(NC):
        d = np.full(LOCAL, 1e30, dtype=np.float32)
        d[:LR] = deg[c * LR:(c + 1) * LR].astype(np.float32)
        degt[c] = d.reshape(NT, 128).T

    # xT [NT, F_IN, 128]
    xT = np.zeros((NC, NT, F_IN, 128), dtype=np.float32)
    xs = np.asarray(x, dtype=np.float32)
    for c in range(NC):
        xl = np.zeros((LOCAL, F_IN), dtype=np.float32)
        xl[:LR] = xs[c * LR:(c + 1) * LR]
        xT[c] = xl.reshape(NT, 128, F_IN).transpose(0, 2, 1)

    meta = dict(N=N, F_IN=F_IN, NC=NC, LR=LR, LOCAL=LOCAL, NT=NT,
                GLOBAL=GLOBAL, GTAB=NWIN * WR, QL=QL,
                NWIN=NWIN, WR=WR, DW=DW, off1=off1)
    data = dict(idx1=idx1, idx2=idx2, degt=degt, xT=xT)
    return meta, data


def make_in_maps(meta, data, W1, b1, W2, b2, W3, b3):
    NC = meta["NC"]
    F_H = W1.shape[1]
    F_OUT = W3.shape[1]
    in_maps = []
    for c in range(NC):
        m = {
            "xT": data["xT"][c],
            "degt": data["degt"][c],
            "W1": np.asarray(W1, np.float32),
            "W2": np.asarray(W2, np.float32),
            "W3": np.asarray(W3, np.float32),
            "b1": np.tile(np.asarray(b1, np.float32), (128, 1)),
            "b2": np.tile(np.asarray(b2, np.float32), (128, 1)),
            "b3": np.tile(np.asarray(b3, np.float32), (128, 1)),
            "idx2": data["idx2"][c],
        }
        for w in range(meta["NWIN"]):
            m[f"idx1_{w}"] = data["idx1"][c][w]
        in_maps.append(m)
    return in_maps


def build(meta, F_H, F_OUT, fake_cc=False, ablate=(), nq=4, gbufs=6,
          gcap=24, paired=True):
    """ablate: subset of {'s1gather','s1reduce','s2','phaseA','cc'} to SKIP
    (timing experiments only; output is garbage when used)."""
    N = meta["N"]; F_IN = meta["F_IN"]; NC = meta["NC"]
    LOCAL = meta["LOCAL"]; NT = meta["NT"]; GTAB = meta["GTAB"]
    QL = meta["QL"]
    NWIN = meta["NWIN"]; WR = meta["WR"]; DW = meta["DW"]; off1 = meta["off1"]
    CH2 = 7 if NT % 7 == 0 else (2 if NT % 2 == 0 else 1)
    CHN = LOCAL // CH2            # idxs per stage-2 chunk
    assert CHN % 128 == 0
    CHT = NT // CH2               # tiles per stage-2 chunk
    f32 = mybir.dt.float32

    NQ = nq  # SWDGE queues (ucode max 4; more queues -> more gather thpt)
    nc = bacc.Bacc("TRN2", target_bir_lowering=False, debug=False,
                   num_devices=NC, num_swdge_queues=NQ)

    xT_d = nc.dram_tensor("xT", [NT, F_IN, 128], f32, kind="ExternalInput")
    degt_d = nc.dram_tensor("degt", [128, NT], f32, kind="ExternalInput")
    W1_d = nc.dram_tensor("W1", [F_IN, F_H], f32, kind="ExternalInput")
    W2_d = nc.dram_tensor("W2", [F_H, F_H], f32, kind="ExternalInput")
    W3_d = nc.dram_tensor("W3", [F_H, F_OUT], f32, kind="ExternalInput")
    b1_d = nc.dram_tensor("b1", [128, F_H], f32, kind="ExternalInput")
    b2_d = nc.dram_tensor("b2", [128, F_H], f32, kind="ExternalInput")
    b3_d = nc.dram_tensor("b3", [128, F_OUT], f32, kind="ExternalInput")
    idx1_d = [nc.dram_tensor(f"idx1_{w}", [128, int(off1[w, NT])], mybir.dt.int16,
                             kind="ExternalInput") for w in range(NWIN)]
    idx2_d = nc.dram_tensor("idx2", [NWIN, 128, LOCAL // 16], mybir.dt.int16,
                            kind="ExternalInput")
    out_d = nc.dram_tensor("out", [128, NT * F_OUT], f32, kind="ExternalOutput")

    with tile.TileContext(nc) as tc:
        with tc.tile_pool(name="const", bufs=1) as constp, \
             tc.tile_pool(name="persist", bufs=1) as persist, \
             tc.tile_pool(name="work", bufs=3) as work, \
             tc.tile_pool(name="gbuf", bufs=gbufs) as gbuf, \
             tc.tile_pool(name="s2p", bufs=3) as s2p, \
             tc.tile_pool(name="idxw", bufs=2) as idxw, \
             tc.tile_pool(name="psum", bufs=2, space="PSUM") as psum, \
             tc.tile_pool(name="dram", bufs=1, space="DRAM") as dram:

            ident = constp.tile([128, 128], f32)
            make_identity(nc, ident[:])
            W1t = constp.tile([F_IN, F_H], f32)
            nc.sync.dma_start(out=W1t[:], in_=W1_d[:])
            W2t = constp.tile([F_H, F_H], f32)
            nc.sync.dma_start(out=W2t[:], in_=W2_d[:])
            # W2 duplicated in both partition halves (paired-transpose mms)
            W2s = constp.tile([128, F_H], f32)
            nc.sync.dma_start(out=W2s[0:F_H, :], in_=W2_d[:])
            nc.sync.dma_start(out=W2s[F_H:2 * F_H, :], in_=W2_d[:])
            W3t = constp.tile([F_H, F_OUT], f32)
            nc.sync.dma_start(out=W3t[:], in_=W3_d[:])
            # W3 duplicated in both partition halves (PE needs lhsT/rhs at
            # the same base partition for the paired-transpose matmuls)
            W3s = constp.tile([128, F_OUT], f32)
            nc.sync.dma_start(out=W3s[0:F_H, :], in_=W3_d[:])
            nc.sync.dma_start(out=W3s[F_H:2 * F_H, :], in_=W3_d[:])
            b1t = constp.tile([128, F_H], f32)
            nc.sync.dma_start(out=b1t[:], in_=b1_d[:])
            b2t = constp.tile([128, F_H], f32)
            nc.sync.dma_start(out=b2t[:], in_=b2_d[:])
            b3t = constp.tile([128, F_OUT], f32)
            nc.sync.dma_start(out=b3t[:], in_=b3_d[:])

            degt = constp.tile([128, NT], f32)
            nc.sync.dma_start(out=degt[:], in_=degt_d[:])
            rdeg = constp.tile([128, NT], f32)
            nc.vector.reciprocal(rdeg[:], degt[:])
            dinv = constp.tile([128, NT], f32)
            nc.scalar.sqrt(dinv[:], rdeg[:])

            do_pA = 'phaseA' not in ablate
            do_cc = 'cc' not in ablate
            do_g1 = 's1gather' not in ablate
            do_r1 = 's1reduce' not in ablate
            do_s2 = 's2' not in ablate

            h = persist.tile([128, NT * F_H], f32)      # layer activations
            acc = persist.tile([128, NT * F_H], f32)    # stage-2 accumulator
            if not do_s2:
                nc.vector.memset(acc[:], 0.0)
            if not do_pA:
                nc.vector.memset(h[:], 0.0)

            slice_t = dram.tile([LOCAL, F_H], f32)      # this core's p slice
            tables = [dram.tile([GTAB, F_H], f32, name=f"table{i}") for i in range(3)]
            # +128 rows slack: 512B paired fetches of row r also read row r+1
            Tw = [dram.tile([LOCAL + 128, F_H], f32, name=f"Tw{i}")
                  for i in range(NWIN)]

            zt = constp.tile([128, F_H], f32)
            nc.vector.memset(zt[:], 0.0)
            for tab in tables:
                for w in range(NWIN):
                    nc.sync.dma_start(
                        out=tab[w * WR + NC * QL:w * WR + NC * QL + 128, :],
                        in_=zt[:])

            gq = [0]  # global pool-DMA (gather) counter for queue/lane alignment
            for L in range(3):
                table = tables[L]
                # ---- Phase A: p slice = dinv * (h @ W)  (L3: dinv * h) ----
                if L == 1 and do_pA:
                    # one [128,128] transpose+copy per PAIR of tiles (same
                    # trick as the L3 head); W2 staged in both partition
                    # halves so lhsT/rhs base partitions match
                    for t2 in range(NT // 2):
                        ptr = psum.tile([128, 128], f32, tag="tr")
                        nc.tensor.transpose(
                            ptr[:], h[:, t2 * 2 * F_H:(t2 + 1) * 2 * F_H],
                            ident[:])
                        hT2 = work.tile([128, 128], f32, tag="lhsT2")
                        nc.scalar.copy(hT2[:], ptr[:])
                        for s_ in range(2):
                            t = 2 * t2 + s_
                            ptile = work.tile([128, F_H], f32, tag="ptile")
                            pm = psum.tile([128, F_H], f32, tag="mm")
                            nc.tensor.matmul(
                                pm[:], lhsT=hT2[s_ * F_H:(s_ + 1) * F_H, :],
                                rhs=W2s[s_ * F_H:(s_ + 1) * F_H, :],
                                start=True, stop=True)
                            nc.scalar.activation(
                                ptile[:], pm[:],
                                mybir.ActivationFunctionType.Copy,
                                scale=dinv[:, t:t + 1])
                            nc.sync.dma_start(
                                out=slice_t[t * 128:(t + 1) * 128, :],
                                in_=ptile[:])
                for t in range(NT if (do_pA and L != 1) else 0):
                    ptile = work.tile([128, F_H], f32, tag="ptile")
                    if L == 0:
                        lhsT = work.tile([F_IN, 128], f32, tag="lhsT")
                        nc.sync.dma_start(out=lhsT[:], in_=xT_d[t])
                        pm = psum.tile([128, F_H], f32, tag="mm")
                        nc.tensor.matmul(pm[:], lhsT=lhsT[:], rhs=W1t[:],
                                         start=True, stop=True)
                        nc.scalar.activation(
                            ptile[:], pm[:],
                            mybir.ActivationFunctionType.Copy,
                            scale=dinv[:, t:t + 1])
                    else:
                        nc.scalar.activation(
                            ptile[:], h[:, t * F_H:(t + 1) * F_H],
                            mybir.ActivationFunctionType.Copy,
                            scale=dinv[:, t:t + 1])
                    nc.sync.dma_start(out=slice_t[t * 128:(t + 1) * 128, :],
                                      in_=ptile[:])

                for q in range(NWIN if do_cc else 0):
                    if fake_cc:
                        for i in range(NC):
                            nc.sync.dma_start(
                                out=table[q * WR + i * QL:q * WR + (i + 1) * QL, :],
                                in_=slice_t[q * QL:(q + 1) * QL, :])
                    else:
                        nc.gpsimd.collective_compute(
                            "AllGather", mybir.AluOpType.bypass,
                            replica_groups=[list(range(NC))],
                            ins=[slice_t[q * QL:(q + 1) * QL, :].opt()],
                            outs=[table[q * WR:q * WR + NC * QL, :].opt()],
                        )

                # ---- Stage 1 + 2 interleaved per window ----
                # Window-outer: window w's stage-2 combine overlaps window
                # w+1's stage-1 gathers. Per window, the whole idx list is
                # preloaded once (one big DMA instead of ~10 on the critical
                # path). queue_num follows the global pool-DMA counter so
                # Tile's round-robin DMASW lane (i%8) always lands on queue
                # i%2.
                F2 = 2 * F_H if paired else F_H  # fetch width per idx
                GCAP = gcap  # max sum of D per gather call
                for w in range(NWIN):
                    wcols = int(off1[w, NT])
                    iw = idxw.tile([128, wcols], mybir.dt.int16, tag="idx1w")
                    nc.sync.dma_start(out=iw[:], in_=idx1_d[w][:])
                    # overlapping-row source view: [WR-64, 128] @ row-stride 64
                    src1 = table[:][w * WR:(w + 1) * WR, :]
                    if paired:
                        src1 = _with_ap(src1, [[F_H, WR - F_H], [1, F2]])
                    assert NT % 2 == 0
                    rw = None
                    t0 = 0
                    while t0 < NT:
                        t1 = t0 + 1
                        sumD = int(DW[w, t0])
                        while t1 < NT and sumD + int(DW[w, t1]) <= GCAP:
                            sumD += int(DW[w, t1])
                            t1 += 1
                        NI = 128 * sumD
                        g = gbuf.tile([128, sumD * F2], f32, tag="g")
                        if do_g1:
                            nc.gpsimd.dma_gather(
                                g[:].rearrange("p (k f) -> p k f", f=F2),
                                src1,
                                iw[:, int(off1[w, t0]):int(off1[w, t1])],
                                NI, NI, F2, elem_step=F_H,
                                single_packet=False, queue_num=gq[0] % NQ)
                            gq[0] += 1
                        off = 0
                        for t in range(t0, t1 if do_r1 else t0):
                            D = int(DW[w, t])
                            # reduce outputs packed in tile pairs: bucket row
                            # (t, p) stored at Tw row (t//2)*256 + 2p + t%2,
                            # so each pair writes 512B/partition runs
                            if t % 2 == 0:
                                rw = s2p.tile([128, 2 * F_H], f32, tag="redw")
                            r_out = rw[:, (t % 2) * F_H:(t % 2 + 1) * F_H]
                            gv = g[:, off * F2:(off + D) * F2].rearrange(
                                "p (k f) -> p f k", f=F2)
                            if paired:
                                pat = [list(p) for p in gv.ap]
                                pat[1] = [1, F_H]   # left half of each pair
                                gv = _with_ap(gv, pat)
                            nc.vector.tensor_reduce(
                                out=r_out,
                                in_=gv,
                                axis=mybir.AxisListType.X,
                                op=mybir.AluOpType.add)
                            if t % 2 == 1:
                                base = Tw[w][(t // 2) * 256:(t // 2 + 1) * 256, :]
                                nc.sync.dma_start(
                                    out=_with_ap(base,
                                                 [[2 * F_H, 128], [1, 2 * F_H]]),
                                    in_=rw[:])
                            off += D
                        t0 = t1
                    # stage-2 combine for this window (paired 512B fetches)
                    src2 = Tw[w][:]
                    if paired:
                        src2 = _with_ap(src2, [[F_H, LOCAL], [1, F2]])
                    if do_s2:
                        it2 = work.tile([128, LOCAL // 16], mybir.dt.int16,
                                        tag="idx2w")
                        nc.sync.dma_start(out=it2[:], in_=idx2_d[w][:])
                    for k in range(CH2 if do_s2 else 0):
                        s2 = s2p.tile([128, CHT * F2], f32, tag="s2")
                        nc.gpsimd.dma_gather(
                            s2[:].rearrange("p (c f) -> p c f", f=F2),
                            src2,
                            it2[:, k * (CHN // 16):(k + 1) * (CHN // 16)],
                            CHN, CHN, F2, elem_step=F_H,
                            single_packet=False, queue_num=gq[0] % NQ)
                        gq[0] += 1
                        a = acc[:, k * CHT * F_H:(k + 1) * CHT * F_H]
                        av = a.rearrange("p (c f) -> p c f", f=F_H)
                        sv = s2[:].rearrange("p (c f) -> p c f", f=F2)
                        if paired:
                            pat = [list(p) for p in sv.ap]
                            pat[2] = [1, F_H]   # left half of each pair
                            sv = _with_ap(sv, pat)
                        if w == 0:
                            nc.vector.tensor_copy(av, sv)
                        else:
                            nc.vector.tensor_tensor(out=av, in0=av, in1=sv,
                                                    op=mybir.AluOpType.add)

                # ---- Epilogue (batched over EB-tile chunks) ----
                EB = 14 if NT % 14 == 0 else (2 if NT % 2 == 0 else 1)
                if L < 2:
                    bt = b1t if L == 0 else b2t
                    for te in range(0, NT, EB):
                        a = acc[:, te * F_H:(te + EB) * F_H]
                        hs = h[:, te * F_H:(te + EB) * F_H]
                        av = a.rearrange("p (t f) -> p t f", f=F_H)
                        hv = hs.rearrange("p (t f) -> p t f", f=F_H)
                        dv = dinv[:, te:te + EB]
                        dbc = _with_ap(dv, [list(dv.ap[0]), [1, EB], [0, F_H]])
                        btv = bt[:]
                        bbc = _with_ap(btv, [list(btv.ap[0]), [0, EB], [1, F_H]])
                        nc.vector.tensor_tensor(out=hv, in0=av, in1=dbc,
                                                op=mybir.AluOpType.mult)
                        nc.vector.tensor_tensor(out=hv, in0=hv, in1=bbc,
                                                op=mybir.AluOpType.add)
                        nc.vector.tensor_scalar_max(out=hs, in0=hs, scalar1=0.0)
                else:
                    outt = persist.tile([128, NT * F_OUT], f32)
                    # z = dinv*agg, two tiles at a time: one transpose+copy
                    # per pair, then per-tile matmul into outt (with bias)
                    for t2 in range(NT // 2):
                        a2 = acc[:, t2 * 2 * F_H:(t2 + 1) * 2 * F_H]
                        z2 = work.tile([128, 2 * F_H], f32, tag="z2")
                        av = a2.rearrange("p (t f) -> p t f", f=F_H)
                        zv = z2[:].rearrange("p (t f) -> p t f", f=F_H)
                        dv = dinv[:, 2 * t2:2 * t2 + 2]
                        dbc = _with_ap(dv, [list(dv.ap[0]), [1, 2], [0, F_H]])
                        nc.vector.tensor_tensor(out=zv, in0=av, in1=dbc,
                                                op=mybir.AluOpType.mult)
                        ptr = psum.tile([128, 128], f32, tag="tr")
                        nc.tensor.transpose(ptr[:], z2[:], ident[:])
                        zT2 = work.tile([128, 128], f32, tag="lhsT2")
                        nc.scalar.copy(zT2[:], ptr[:])
                        for s_ in range(2):
                            t = 2 * t2 + s_
                            pm = psum.tile([128, F_OUT], f32, tag="mmo")
                            nc.tensor.matmul(
                                pm[:], lhsT=zT2[s_ * F_H:(s_ + 1) * F_H, :],
                                rhs=W3s[s_ * F_H:(s_ + 1) * F_H, :],
                                start=True, stop=True)
                            nc.vector.tensor_tensor(
                                out=outt[:, t * F_OUT:(t + 1) * F_OUT],
                                in0=pm[:], in1=b3t[:],
                                op=mybir.AluOpType.add)
                    # log_softmax batched over EB-tile chunks
                    for te in range(0, NT, EB):
                        ov = outt[:, te * F_OUT:(te + EB) * F_OUT]
                        ov3 = ov.rearrange("p (t f) -> p t f", f=F_OUT)
                        mx = work.tile([128, EB], f32, tag="mx")
                        nc.vector.tensor_reduce(out=mx[:], in_=ov3,
                                                axis=mybir.AxisListType.X,
                                                op=mybir.AluOpType.max,
                                                negate=True)  # -max per (p,t)
                        mbc = _with_ap(mx[:],
                                       [list(mx[:].ap[0]), [1, EB], [0, F_OUT]])
                        nc.vector.tensor_tensor(out=ov3, in0=ov3, in1=mbc,
                                                op=mybir.AluOpType.add)
                        e = work.tile([128, EB * F_OUT], f32, tag="e")
                        nc.scalar.activation(e[:], ov,
                                             mybir.ActivationFunctionType.Exp)
                        sm = work.tile([128, EB], f32, tag="s")
                        nc.vector.tensor_reduce(
                            out=sm[:],
                            in_=e[:].rearrange("p (t f) -> p t f", f=F_OUT),
                            axis=mybir.AxisListType.X,
                            op=mybir.AluOpType.add)
                        ls = work.tile([128, EB], f32, tag="ls")
                        nc.scalar.activation(ls[:], sm[:],
                                             mybir.ActivationFunctionType.Ln)
                        lbc = _with_ap(ls[:],
                                       [list(ls[:].ap[0]), [1, EB], [0, F_OUT]])
                        nc.vector.tensor_tensor(out=ov3, in0=ov3, in1=lbc,
                                                op=mybir.AluOpType.subtract)
                    nc.sync.dma_start(out=out_d[:], in_=outt[:])

    nc.compile()
    return nc


def postprocess(meta, results, F_OUT):
    """results: list of per-core {"out": [128, NT*F_OUT]} -> [N, F_OUT]."""
    N = meta["N"]; NC = meta["NC"]; LR = meta["LR"]; NT = meta["NT"]
    full = np.zeros((N, F_OUT), dtype=np.float32)
    for c in range(NC):
        o = results[c]["out"].reshape(128, NT, F_OUT)
        o = o.transpose(1, 0, 2).reshape(NT * 128, F_OUT)
        full[c * LR:(c + 1) * LR] = o[:LR]
    return full


_CACHE = {}


def kernel(**inputs):
    """Full-input entry: shards across 8 NeuronCores internally."""
    from concourse import bass_utils

    x = np.asarray(inputs["x"], dtype=np.float32)
    edge_index = np.asarray(inputs["edge_index"])
    W1 = np.asarray(inputs["W1"], np.float32)
    W2 = np.asarray(inputs["W2"], np.float32)
    W3 = np.asarray(inputs["W3"], np.float32)
    b1 = np.asarray(inputs["b1"], np.float32)
    b2 = np.asarray(inputs["b2"], np.float32)
    b3 = np.asarray(inputs["b3"], np.float32)
    F_H, F_OUT = W1.shape[1], W3.shape[1]

    import hashlib
    key = (x.shape, edge_index.shape, F_H, F_OUT,
           hashlib.sha1(np.ascontiguousarray(edge_index)).hexdigest())
    meta, data = preprocess(x, edge_index, n_cores=8)
    if key in _CACHE:
        nc = _CACHE[key]
    else:
        nc = build(meta, F_H, F_OUT)
        _CACHE[key] = nc

    in_maps = make_in_maps(meta, data, W1, b1, W2, b2, W3, b3)
    res = bass_utils.run_bass_kernel_spmd(
        nc, in_maps, core_ids=list(range(meta["NC"])))
    return postprocess(meta, res.results, F_OUT)

